# revision 4
# baseline (speedup 1.0000x reference)
"""Trainium2 Bass kernel for StyleGAN2-style 4x4 blur (upfirdn2d, up=down=1,
pad=(2,1)) on x:[8,128,256,256] fp32.

Math: out[i,j] = sum_{p,q in [-2,1]} K[1-p,1-q] * x[i+p, j+q]  (zero-padded),
with K the 4x4 blur kernel. K is rank-1 (outer product), so the conv is
separable: an H-pass with taps from the column factor and a W-pass with taps
from the row factor.

Mapping to hardware: each 1-D conv is a banded-matrix product. Per (b,c)
image (256x256) we run two PSUM-accumulated matmul groups on TensorE using
float32r (relaxed fp32, full-rate at N>=256):

  MM1:  t1[w, h'] = sum_h x[h, w] * BH[h, h']      (H-conv, output transposed)
  MM2:  y[h', w'] = sum_w t1[w, h'] * BW[w, w']    (W-conv, transposes back)

K (contraction) is capped at 128, so each group is 2 accumulating matmuls
over 128-row halves; the 256-wide bands fold the zero padding at the image
borders. float32r keeps fp32 storage (no cast DMAs) at ~tf32 multiply
precision -- measured rel. error vs the fp32 reference ~2e-4. ScalarE and
VectorE evacuate PSUM->SBUF; HWDGE DMAs on both rings stream HBM.

The kernel is DMA-bound: 33.5 MB in + 33.5 MB out per core. Measured
~208 us/core (~90% of the ~187 us HBM roofline); compute fully overlaps.
DMA pattern choices that mattered (measured by dT/dR slope over a hardware
repeat loop, which cancels the ~5 ms axon launch overhead): partition p
holds row pair (2p, 2p+1) so every DMA line is 2KB contiguous (the band
matrix absorbs the permutation), 2 images per dma_start, in/out alternating
across the sync/scalar HWDGE rings, deep tile pools (12 input / 8 output
buffers) to keep enough DMAs in flight.

Sharding: batch dim (8) -> one NeuronCore each; channels (128) map to
sequential images per core.
"""

import os
import sys

sys.path.insert(0, "/opt/trn_rl_repo")

import numpy as np

# DMA layout: "v2" = row-pair interleave (2KB contiguous lines),
# "v1" = half-split (two 1KB chunks per line)
LAYOUT = os.environ.get("BLUR_LAYOUT", "v2")

B, C, H, W = 8, 128, 256, 256
KH = KW = 4
N_CORES = 8


def _band_256(taps):
    """Band matrix Bd[k, n] = taps[1 + n - k] for 0 <= 1+n-k < 4, else 0.

    t_out[n] = sum_k Bd[k, n] * x_in[k] is the 1-D conv
    out[n] = sum_{p=-2..1} taps_coeff[p] x[n+p] with taps_coeff[p] = taps[1-p]
    and zero padding (2 leading, 1 trailing) folded in by truncation.
    """
    Bd = np.zeros((256, 256), dtype=np.float64)
    for n in range(256):
        for d in range(4):
            k = n + 1 - d
            if 0 <= k < 256:
                Bd[k, n] = taps[d]
    return Bd


def _factor_kernel(k2):
    """Rank-1 factorization k2 = outer(u, v) (k2 is an outer product)."""
    k2 = np.asarray(k2, dtype=np.float64)
    uu, ss, vv = np.linalg.svd(k2)
    assert ss[1] < 1e-5 * max(ss[0], 1e-30), "blur kernel is not rank-1"
    u = uu[:, 0] * np.sqrt(ss[0])
    v = vv[0] * np.sqrt(ss[0])
    # fix sign so that outer(u, v) ~ k2 with u mostly positive
    if u.sum() < 0:
        u, v = -u, -v
    return u, v


def _make_bands(k2, layout=None):
    """Returns (bh_sb, bw_sb) as float32 [128, 512] SBUF layouts.

    bh_sb[p, j*256 + n] = BH[2p + j, n] -- input rows interleaved in pairs so
    every DMA partition line is one 2KB-contiguous DRAM chunk (rows 2p, 2p+1).
    bw_sb[p, wb*256 + n] = BW[wb*128 + p, n] -- plain half split (W stays on
    partitions of the intermediate, untouched by the interleave).
    """
    if layout is None:
        layout = LAYOUT
    u, v = _factor_kernel(k2)
    # coefficient of x[i+p] is u[1-p] -> band entry BH[k, n] = u[1 + n - k]
    BH = _band_256(u)
    BW = _band_256(v)
    bw_sb = (
        BW.reshape(2, 128, 256).transpose(1, 0, 2).reshape(128, 512)
    ).astype(np.float32)
    if layout == "v2":
        # permute BH's output columns even/odd so MM2 can pick h' = 2i + par
        # with a contiguous 128-col block: column (par*128+i) holds h'=2i+par
        perm = np.concatenate([np.arange(0, 256, 2), np.arange(1, 256, 2)])
        BH = BH[:, perm]
        bh_sb = BH.reshape(128, 2, 256).reshape(128, 512).astype(np.float32)
    else:
        bh_sb = (
            BH.reshape(2, 128, 256).transpose(1, 0, 2).reshape(128, 512)
        ).astype(np.float32)
    return bh_sb, bw_sb


def _make_bands_f16(k2):
    """v1-layout band matrices in fp16: b[p, j*256 + n] = Bd[j*128 + p, n].

    The blur taps (eighths) are exact in fp16, so no precision is lost in
    the band matrices themselves.
    """
    u, v = _factor_kernel(k2)
    BH = _band_256(u)
    BW = _band_256(v)

    def v1(Bd):
        return (
            Bd.reshape(2, 128, 256).transpose(1, 0, 2).reshape(128, 512)
        ).astype(np.float16)

    return v1(BH), v1(BW)


def _build_nc_f16(
    n_images,
    repeats=1,
    mode="band",
    gsz=4,
    bufs=(6, 4, 6, 3, 3),
    alt_rings=True,
    copysplit=False,
):
    """fp16-I/O build: x,y in DRAM as fp16 (half the HBM traffic of fp32),
    v1 row-halves layout (partition p holds rows p and 128+p of each image;
    every DMA chunk is 512B, the smallest size with full DMA efficiency).

    mode="band": the 1-D convs are banded matrices, so each matmul's moving
    operand (the band) is truncated to its nonzero column range. Per w-half,
    output cols [0,127) come only from input rows 0..127 (j=0), cols
    [130,256) only from rows 128..255 (j=1), and only the 3-col boundary
    region [127,130) accumulates both halves. 2*259 moving rows per conv
    pass per image instead of 2*512 -- PE time halves.

    mode="dense": same fp16 I/O but full 256-col matmuls (A/B lever).
    mode="dmaonly": pure DMA in/out roundtrip (measures the traffic floor).
    """
    import contextlib

    import concourse.bacc as bacc
    import concourse.mybir as mybir
    from concourse.tile import TileContext

    f32 = mybir.dt.float32
    f16 = mybir.dt.float16

    nc = bacc.Bacc("TRN2", target_bir_lowering=False)
    x = nc.dram_tensor("x", (n_images, 256, 256), f16, kind="ExternalInput")
    bh = nc.dram_tensor("bh", (128, 512), f16, kind="ExternalInput")
    bw = nc.dram_tensor("bw", (128, 512), f16, kind="ExternalInput")
    y = nc.dram_tensor("y", (n_images, 256, 256), f16, kind="ExternalOutput")

    # partition p holds rows p (j=0) and 128+p (j=1): contract dim = row
    # halves, which is what makes the band truncation work
    x_v = x.rearrange("(cc c2) (j p) w -> cc p c2 j w", c2=gsz, p=128)
    y_v = y.rearrange("(cc c2) (j p) w -> cc p c2 j w", c2=gsz, p=128)

    xt_b, t1_b, yt_b, ps1_b, ps2_b = bufs
    with TileContext(nc) as tc:
        with (
            tc.tile_pool(name="consts", bufs=1) as cpool,
            tc.tile_pool(name="xt", bufs=xt_b) as xpool,
            tc.tile_pool(name="t1", bufs=t1_b) as tpool,
            tc.tile_pool(name="yt", bufs=yt_b) as ypool,
            tc.tile_pool(name="ps1", bufs=ps1_b, space="PSUM") as ps1pool,
            tc.tile_pool(name="ps2", bufs=ps2_b, space="PSUM") as ps2pool,
        ):
            bh_sb = cpool.tile([128, 512], f16, tag="bh")
            bw_sb = cpool.tile([128, 512], f16, tag="bw")
            nc.sync.dma_start(out=bh_sb[:], in_=bh[:])
            nc.sync.dma_start(out=bw_sb[:], in_=bw[:])

            def band_pass(ps, base, lhs0, lhs1, b_sb):
                """One banded 1-D conv half: out cols [base, base+256) of ps.

                lhs0/lhs1: the two 128-contract stationary blocks (input row
                halves). b_sb: the band matrix, v1 layout [128, 2*256].
                Band reach is +-2/+1 rows, so cols [0,127) need only lhs0,
                [130,256) only lhs1, and [127,130) accumulates both.
                """
                nc.tensor.matmul(
                    ps[:, base : base + 127],
                    lhs0,
                    b_sb[:, 0:127],
                    start=True,
                    stop=True,
                )
                nc.tensor.matmul(
                    ps[:, base + 127 : base + 130],
                    lhs0,
                    b_sb[:, 127:130],
                    start=True,
                    stop=False,
                )
                nc.tensor.matmul(
                    ps[:, base + 127 : base + 130],
                    lhs1,
                    b_sb[:, 256 + 127 : 256 + 130],
                    start=False,
                    stop=True,
                )
                nc.tensor.matmul(
                    ps[:, base + 130 : base + 256],
                    lhs1,
                    b_sb[:, 256 + 130 : 512],
                    start=True,
                    stop=True,
                )

            loop_ctx = (
                tc.For_i(0, repeats, 1) if repeats > 1 else contextlib.nullcontext()
            )
            with loop_ctx:
                for cc in range(n_images // gsz):
                    in_eng = nc.sync if (not alt_rings or cc % 2 == 0) else nc.scalar
                    out_eng = nc.scalar if (not alt_rings or cc % 2 == 0) else nc.sync
                    xt = xpool.tile([128, 512 * gsz], f16)
                    in_eng.dma_start(
                        out=xt[:].rearrange("p (c2 j w) -> p c2 j w", c2=gsz, j=2),
                        in_=x_v[cc],
                    )
                    if mode == "dmaonly":
                        out_eng.dma_start(
                            out=y_v[cc],
                            in_=xt[:].rearrange(
                                "p (c2 j w) -> p c2 j w", c2=gsz, j=2
                            ),
                        )
                        continue

                    yt = ypool.tile([128, 512 * gsz], f16)
                    for c2 in range(gsz):
                        xo = c2 * 512

                        # MM1: t1[w, h'] = sum_h x[h, w] * BH[h, h']
                        ps1 = ps1pool.tile([128, 512], f32)
                        for wb in range(2):
                            xs0 = xt[:, xo + wb * 128 : xo + wb * 128 + 128]
                            xs1 = xt[:, xo + 256 + wb * 128 : xo + 256 + wb * 128 + 128]
                            if mode == "band":
                                band_pass(ps1, wb * 256, xs0, xs1, bh_sb)
                            else:
                                for j, lhsT in ((0, xs0), (1, xs1)):
                                    nc.tensor.matmul(
                                        ps1[:, wb * 256 : (wb + 1) * 256],
                                        lhsT,
                                        bh_sb[:, j * 256 : (j + 1) * 256],
                                        start=(j == 0),
                                        stop=(j == 1),
                                    )

                        t1 = tpool.tile([128, 512], f16)
                        if copysplit:
                            nc.scalar.copy(out=t1[:, 0:256], in_=ps1[:, 0:256])
                            nc.vector.tensor_copy(
                                out=t1[:, 256:512], in_=ps1[:, 256:512]
                            )
                        else:
                            nc.scalar.copy(out=t1[:], in_=ps1[:])

                        # MM2: y[h', w'] = sum_w t1[w, h'] * BW[w, w']
                        # ps2[p, par*256+w'] = y[par*128+p, w'] -> v1 output
                        ps2 = ps2pool.tile([128, 512], f32)
                        for par in range(2):
                            t1s0 = t1[:, par * 128 : par * 128 + 128]
                            t1s1 = t1[:, 256 + par * 128 : 256 + par * 128 + 128]
                            if mode == "band":
                                band_pass(ps2, par * 256, t1s0, t1s1, bw_sb)
                            else:
                                for wb, lhsT in ((0, t1s0), (1, t1s1)):
                                    nc.tensor.matmul(
                                        ps2[:, par * 256 : (par + 1) * 256],
                                        lhsT,
                                        bw_sb[:, wb * 256 : (wb + 1) * 256],
                                        start=(wb == 0),
                                        stop=(wb == 1),
                                    )

                        if copysplit:
                            nc.vector.tensor_copy(
                                out=yt[:, xo : xo + 256], in_=ps2[:, 0:256]
                            )
                            nc.scalar.copy(
                                out=yt[:, xo + 256 : xo + 512], in_=ps2[:, 256:512]
                            )
                        else:
                            nc.vector.tensor_copy(
                                out=yt[:, xo : xo + 512], in_=ps2[:]
                            )

                    out_eng.dma_start(
                        out=y_v[cc],
                        in_=yt[:].rearrange("p (c2 j w) -> p c2 j w", c2=gsz, j=2),
                    )

    nc.compile()
    return nc


_NC_CACHE = {}


def _build_nc(n_images, repeats=1, mode="full", layout=None, gsz=2,
              bufs=(12, 4, 8, 3, 3), alt_rings=True, swdge_in=False,
              tri=False, copysplit=False, burst=0):
    """Builds the per-core Bass module.

    gsz: images per input/output DMA (bigger transfers, fewer instructions)
    bufs: (xt, t1, yt, ps1, ps2) tile-pool buffer counts
    alt_rings: alternate in/out DMAs across both HWDGE rings (sync/scalar)
    """
    if layout is None:
        layout = LAYOUT
    import contextlib

    import concourse.bacc as bacc
    import concourse.mybir as mybir
    from concourse.tile import TileContext

    f32 = mybir.dt.float32
    f32r = mybir.dt.float32r

    nc = bacc.Bacc("TRN2", target_bir_lowering=False)
    x = nc.dram_tensor("x", (n_images, 256, 256), f32r, kind="ExternalInput")
    bh = nc.dram_tensor("bh", (128, 512), f32r, kind="ExternalInput")
    bw = nc.dram_tensor("bw", (128, 512), f32r, kind="ExternalInput")
    y = nc.dram_tensor("y", (n_images, 256, 256), f32, kind="ExternalOutput")

    if layout == "v2":
        # partition p holds rows 2p and 2p+1: 2KB-contiguous DMA lines
        x_v = x.rearrange("(cc c2) (p j) w -> cc p c2 j w", c2=gsz, j=2)
        y_v = y.rearrange("(cc c2) (p j) w -> cc p c2 j w", c2=gsz, j=2)
    else:
        # partition p holds rows p and 128+p: two 1KB chunks per image
        x_v = x.rearrange("(cc c2) (j p) w -> cc p c2 j w", c2=gsz, p=128)
        y_v = y.rearrange("(cc c2) (j p) w -> cc p c2 j w", c2=gsz, p=128)

    xt_b, t1_b, yt_b, ps1_b, ps2_b = bufs
    with TileContext(nc) as tc:
        with (
            tc.tile_pool(name="consts", bufs=1) as cpool,
            tc.tile_pool(name="xt", bufs=xt_b) as xpool,
            tc.tile_pool(name="t1", bufs=t1_b) as tpool,
            tc.tile_pool(name="yt", bufs=yt_b) as ypool,
            tc.tile_pool(name="ps1", bufs=ps1_b, space="PSUM") as ps1pool,
            tc.tile_pool(name="ps2", bufs=ps2_b, space="PSUM") as ps2pool,
        ):
            bh_sb = cpool.tile([128, 512], f32r, tag="bh")
            bw_sb = cpool.tile([128, 512], f32r, tag="bw")
            nc.sync.dma_start(out=bh_sb[:], in_=bh[:])
            nc.sync.dma_start(out=bw_sb[:], in_=bw[:])

            loop_ctx = (
                tc.For_i(0, repeats, 1) if repeats > 1 else contextlib.nullcontext()
            )
            with loop_ctx:
                pending_outs = []
                for cc in range(n_images // gsz):
                    in_eng = nc.sync if (not alt_rings or cc % 2 == 0) else nc.scalar
                    out_eng = nc.scalar if (not alt_rings or cc % 2 == 0) else nc.sync
                    if swdge_in:
                        in_eng = nc.gpsimd
                    if tri:
                        # third DGE path: SWDGE carries half the input stream
                        in_eng = nc.sync if cc % 2 == 0 else nc.gpsimd
                        out_eng = nc.scalar
                    xt = xpool.tile([128, 512 * gsz], f32r)
                    in_eng.dma_start(
                        out=xt[:].rearrange("p (c2 j w) -> p c2 j w", c2=gsz, j=2),
                        in_=x_v[cc],
                    )
                    if mode == "dmaonly":
                        out_eng.dma_start(
                            out=y_v[cc],
                            in_=xt[:]
                            .bitcast(f32)
                            .rearrange("p (c2 j w) -> p c2 j w", c2=gsz, j=2),
                        )
                        continue

                    yt = ypool.tile([128, 512 * gsz], f32)
                    for c2 in range(gsz):
                        xo = c2 * 512
                        # MM1: t1[w, h'] = sum_h x[h, w] * BH[h, h']
                        ps1 = ps1pool.tile([128, 512], f32)
                        for wb in range(2):
                            for j in range(2):
                                lhsT = xt[
                                    :,
                                    xo + j * 256 + wb * 128 : xo
                                    + j * 256
                                    + wb * 128
                                    + 128,
                                ]
                                rhs = bh_sb[:, j * 256 : (j + 1) * 256]
                                nc.tensor.matmul(
                                    ps1[:, wb * 256 : (wb + 1) * 256],
                                    lhsT,
                                    rhs,
                                    start=(j == 0),
                                    stop=(j == 1),
                                )

                        t1 = tpool.tile([128, 512], f32r)
                        if copysplit:
                            nc.scalar.copy(out=t1[:, 0:256], in_=ps1[:, 0:256])
                            nc.vector.tensor_copy(
                                out=t1[:, 256:512], in_=ps1[:, 256:512]
                            )
                        else:
                            nc.scalar.copy(out=t1[:], in_=ps1[:])

                        # MM2: y[h', w'] = sum_w t1[w, h'] * BW[w, w']
                        ps2 = ps2pool.tile([128, 512], f32)
                        for par in range(2):
                            for wb in range(2):
                                lhsT = t1[
                                    :,
                                    wb * 256 + par * 128 : wb * 256 + par * 128 + 128,
                                ]
                                rhs = bw_sb[:, wb * 256 : (wb + 1) * 256]
                                nc.tensor.matmul(
                                    ps2[:, par * 256 : (par + 1) * 256],
                                    lhsT,
                                    rhs,
                                    start=(wb == 0),
                                    stop=(wb == 1),
                                )

                        if copysplit:
                            nc.vector.tensor_copy(
                                out=yt[:, c2 * 512 : c2 * 512 + 256],
                                in_=ps2[:, 0:256],
                            )
                            nc.scalar.copy(
                                out=yt[:, c2 * 512 + 256 : (c2 + 1) * 512],
                                in_=ps2[:, 256:512],
                            )
                        else:
                            nc.vector.tensor_copy(
                                out=yt[:, c2 * 512 : (c2 + 1) * 512], in_=ps2[:]
                            )
                    if burst:
                        pending_outs.append((cc, yt))
                        if len(pending_outs) >= burst:
                            for occ, oyt in pending_outs:
                                nc.scalar.dma_start(
                                    out=y_v[occ],
                                    in_=oyt[:].rearrange(
                                        "p (c2 j w) -> p c2 j w", c2=gsz, j=2
                                    ),
                                )
                            pending_outs = []
                    else:
                        out_eng.dma_start(
                            out=y_v[cc],
                            in_=yt[:].rearrange(
                                "p (c2 j w) -> p c2 j w", c2=gsz, j=2
                            ),
                        )
                for occ, oyt in pending_outs:
                    nc.scalar.dma_start(
                        out=y_v[occ],
                        in_=oyt[:].rearrange("p (c2 j w) -> p c2 j w", c2=gsz, j=2),
                    )

    nc.compile()
    return nc


def _get_nc(n_images, repeats=1, mode="full", layout=None, **kw):
    key = (n_images, repeats, mode, layout or LAYOUT, tuple(sorted(kw.items())))
    if key not in _NC_CACHE:
        if (layout or LAYOUT) == "b16":
            _NC_CACHE[key] = _build_nc_f16(n_images, repeats, mode, **kw)
        else:
            _NC_CACHE[key] = _build_nc(n_images, repeats, mode, layout, **kw)
    return _NC_CACHE[key]


def _make_in_map(x_core, k2, layout=None):
    """Device input map for one core from its [C,H,W] fp32 slice."""
    if layout is None:
        layout = LAYOUT
    if layout == "b16":
        bh_sb, bw_sb = _make_bands_f16(k2)
        return {
            "x": np.ascontiguousarray(x_core, dtype=np.float16),
            "bh": bh_sb,
            "bw": bw_sb,
        }
    bh_sb, bw_sb = _make_bands(k2, layout)
    return {
        "x": np.ascontiguousarray(x_core, dtype=np.float32),
        "bh": bh_sb,
        "bw": bw_sb,
    }


# default device pipeline: fp16 I/O + banded matmuls
MODE = os.environ.get("BLUR_MODE", "band")


def kernel(x, kernel, _trace=False):
    from concourse import bass_utils

    x = np.asarray(x)
    k2 = np.asarray(kernel, dtype=np.float32)
    assert x.shape == (B, C, H, W), x.shape
    assert k2.shape == (KH, KW), k2.shape

    if LAYOUT == "b16":
        x16 = np.ascontiguousarray(x, dtype=np.float16)
        bh_sb, bw_sb = _make_bands_f16(k2)
        nc = _get_nc(C, mode=MODE, layout="b16")
        in_maps = [{"x": x16[b], "bh": bh_sb, "bw": bw_sb} for b in range(B)]
    else:
        x32 = np.ascontiguousarray(x, dtype=np.float32)
        bh_sb, bw_sb = _make_bands(k2)
        nc = _get_nc(C)
        in_maps = [{"x": x32[b], "bh": bh_sb, "bw": bw_sb} for b in range(B)]
    res = bass_utils.run_bass_kernel_spmd(
        nc, in_maps, core_ids=list(range(N_CORES)), trace=_trace
    )
    out = np.stack(
        [np.asarray(res.results[b]["y"], dtype=np.float32) for b in range(B)],
        axis=0,
    )
    if _trace:
        return out, res
    return out



# revision 19
# speedup vs baseline: 200.4347x; 200.4347x over previous
"""Trainium2 Bass kernel for StyleGAN2-style 4x4 blur (upfirdn2d, up=down=1,
pad=(2,1)) on x:[8,128,256,256] fp32.

Math: out[i,j] = sum_{p,q in [-2,1]} K[1-p,1-q] * x[i+p, j+q]  (zero-padded),
with K the 4x4 blur kernel. K is rank-1 (outer product), so the conv is
separable: an H-pass with taps from the column factor and a W-pass with taps
from the row factor.

Mapping to hardware: each 1-D conv is a banded-matrix product. Per (b,c)
image (256x256) we run two PSUM-accumulated matmul groups on TensorE using
float32r (relaxed fp32, full-rate at N>=256):

  MM1:  t1[w, h'] = sum_h x[h, w] * BH[h, h']      (H-conv, output transposed)
  MM2:  y[h', w'] = sum_w t1[w, h'] * BW[w, w']    (W-conv, transposes back)

K (contraction) is capped at 128, so each group is 2 accumulating matmuls
over 128-row halves; the 256-wide bands fold the zero padding at the image
borders. float32r keeps fp32 storage (no cast DMAs) at ~tf32 multiply
precision -- measured rel. error vs the fp32 reference ~2e-4. ScalarE and
VectorE evacuate PSUM->SBUF; HWDGE DMAs on both rings stream HBM.

The kernel is DMA-bound: 33.5 MB in + 33.5 MB out per core. Measured
~208 us/core (~90% of the ~187 us HBM roofline); compute fully overlaps.
DMA pattern choices that mattered (measured by dT/dR slope over a hardware
repeat loop, which cancels the ~5 ms axon launch overhead): partition p
holds row pair (2p, 2p+1) so every DMA line is 2KB contiguous (the band
matrix absorbs the permutation), 2 images per dma_start, in/out alternating
across the sync/scalar HWDGE rings, deep tile pools (12 input / 8 output
buffers) to keep enough DMAs in flight.

Sharding: batch dim (8) -> one NeuronCore each; channels (128) map to
sequential images per core.
"""

import os
import sys

sys.path.insert(0, "/opt/trn_rl_repo")

import numpy as np

# DMA layout: "b16" = fp16 I/O in row-halves layout (default),
# "h16" = fp16 I/O row-pair layout, "v2"/"v1" = legacy fp32 paths
LAYOUT = os.environ.get("BLUR_LAYOUT", "b16")

B, C, H, W = 8, 128, 256, 256
KH = KW = 4
N_CORES = 8


def _band_256(taps):
    """Band matrix Bd[k, n] = taps[1 + n - k] for 0 <= 1+n-k < 4, else 0.

    t_out[n] = sum_k Bd[k, n] * x_in[k] is the 1-D conv
    out[n] = sum_{p=-2..1} taps_coeff[p] x[n+p] with taps_coeff[p] = taps[1-p]
    and zero padding (2 leading, 1 trailing) folded in by truncation.
    """
    Bd = np.zeros((256, 256), dtype=np.float64)
    for n in range(256):
        for d in range(4):
            k = n + 1 - d
            if 0 <= k < 256:
                Bd[k, n] = taps[d]
    return Bd


def _factor_kernel(k2):
    """Rank-1 factorization k2 = outer(u, v) (k2 is an outer product)."""
    k2 = np.asarray(k2, dtype=np.float64)
    uu, ss, vv = np.linalg.svd(k2)
    assert ss[1] < 1e-5 * max(ss[0], 1e-30), "blur kernel is not rank-1"
    u = uu[:, 0] * np.sqrt(ss[0])
    v = vv[0] * np.sqrt(ss[0])
    # fix sign so that outer(u, v) ~ k2 with u mostly positive
    if u.sum() < 0:
        u, v = -u, -v
    return u, v


def _make_bands(k2, layout=None):
    """Returns (bh_sb, bw_sb) as float32 [128, 512] SBUF layouts.

    bh_sb[p, j*256 + n] = BH[2p + j, n] -- input rows interleaved in pairs so
    every DMA partition line is one 2KB-contiguous DRAM chunk (rows 2p, 2p+1).
    bw_sb[p, wb*256 + n] = BW[wb*128 + p, n] -- plain half split (W stays on
    partitions of the intermediate, untouched by the interleave).
    """
    if layout is None:
        layout = LAYOUT
    u, v = _factor_kernel(k2)
    # coefficient of x[i+p] is u[1-p] -> band entry BH[k, n] = u[1 + n - k]
    BH = _band_256(u)
    BW = _band_256(v)
    bw_sb = (
        BW.reshape(2, 128, 256).transpose(1, 0, 2).reshape(128, 512)
    ).astype(np.float32)
    if layout == "v2":
        # permute BH's output columns even/odd so MM2 can pick h' = 2i + par
        # with a contiguous 128-col block: column (par*128+i) holds h'=2i+par
        perm = np.concatenate([np.arange(0, 256, 2), np.arange(1, 256, 2)])
        BH = BH[:, perm]
        bh_sb = BH.reshape(128, 2, 256).reshape(128, 512).astype(np.float32)
    else:
        bh_sb = (
            BH.reshape(2, 128, 256).transpose(1, 0, 2).reshape(128, 512)
        ).astype(np.float32)
    return bh_sb, bw_sb


def _make_bands_f16(k2):
    """v1-layout band matrices in fp16: b[p, j*256 + n] = Bd[j*128 + p, n].

    The blur taps (eighths) are exact in fp16, so no precision is lost in
    the band matrices themselves.
    """
    u, v = _factor_kernel(k2)
    BH = _band_256(u)
    BW = _band_256(v)

    def v1(Bd):
        return (
            Bd.reshape(2, 128, 256).transpose(1, 0, 2).reshape(128, 512)
        ).astype(np.float16)

    return v1(BH), v1(BW)


def _build_nc_f16(
    n_images,
    repeats=1,
    mode="band",
    gsz=4,
    bufs=(6, 4, 6, 3, 3),
    alt_rings=True,
    copysplit=False,
):
    """fp16-I/O build: x,y in DRAM as fp16 (half the HBM traffic of fp32),
    v1 row-halves layout (partition p holds rows p and 128+p of each image;
    every DMA chunk is 512B, the smallest size with full DMA efficiency).

    mode="band": the 1-D convs are banded matrices, so each matmul's moving
    operand (the band) is truncated to its nonzero column range. Per w-half,
    output cols [0,127) come only from input rows 0..127 (j=0), cols
    [130,256) only from rows 128..255 (j=1), and only the 3-col boundary
    region [127,130) accumulates both halves. 2*259 moving rows per conv
    pass per image instead of 2*512 -- PE time halves.

    mode="dense": same fp16 I/O but full 256-col matmuls (A/B lever).
    mode="dmaonly": pure DMA in/out roundtrip (measures the traffic floor).
    """
    import contextlib

    import concourse.bacc as bacc
    import concourse.mybir as mybir
    from concourse.tile import TileContext

    f32 = mybir.dt.float32
    f16 = mybir.dt.float16

    nc = bacc.Bacc("TRN2", target_bir_lowering=False)
    x = nc.dram_tensor("x", (n_images, 256, 256), f16, kind="ExternalInput")
    bh = nc.dram_tensor("bh", (128, 512), f16, kind="ExternalInput")
    bw = nc.dram_tensor("bw", (128, 512), f16, kind="ExternalInput")
    y = nc.dram_tensor("y", (n_images, 256, 256), f16, kind="ExternalOutput")

    # partition p holds rows p (j=0) and 128+p (j=1): contract dim = row
    # halves, which is what makes the band truncation work
    x_v = x.rearrange("(cc c2) (j p) w -> cc p c2 j w", c2=gsz, p=128)
    y_v = y.rearrange("(cc c2) (j p) w -> cc p c2 j w", c2=gsz, p=128)

    xt_b, t1_b, yt_b, ps1_b, ps2_b = bufs
    with TileContext(nc) as tc:
        with (
            tc.tile_pool(name="consts", bufs=1) as cpool,
            tc.tile_pool(name="xt", bufs=xt_b) as xpool,
            tc.tile_pool(name="t1", bufs=t1_b) as tpool,
            tc.tile_pool(name="yt", bufs=yt_b) as ypool,
            tc.tile_pool(name="ps1", bufs=ps1_b, space="PSUM") as ps1pool,
            tc.tile_pool(name="ps2", bufs=ps2_b, space="PSUM") as ps2pool,
        ):
            bh_sb = cpool.tile([128, 512], f16, tag="bh")
            bw_sb = cpool.tile([128, 512], f16, tag="bw")
            nc.sync.dma_start(out=bh_sb[:], in_=bh[:])
            nc.sync.dma_start(out=bw_sb[:], in_=bw[:])

            def band_pass(ps, base, lhs0, lhs1, b_sb):
                """One banded 1-D conv half: out cols [base, base+256) of ps.

                lhs0/lhs1: the two 128-contract stationary blocks (input row
                halves). b_sb: the band matrix, v1 layout [128, 2*256].
                Band reach is +-2/+1 rows, so cols [0,127) need only lhs0,
                [130,256) only lhs1, and [127,130) accumulates both.
                """
                nc.tensor.matmul(
                    ps[:, base : base + 127],
                    lhs0,
                    b_sb[:, 0:127],
                    start=True,
                    stop=True,
                )
                nc.tensor.matmul(
                    ps[:, base + 127 : base + 130],
                    lhs0,
                    b_sb[:, 127:130],
                    start=True,
                    stop=False,
                )
                nc.tensor.matmul(
                    ps[:, base + 127 : base + 130],
                    lhs1,
                    b_sb[:, 256 + 127 : 256 + 130],
                    start=False,
                    stop=True,
                )
                nc.tensor.matmul(
                    ps[:, base + 130 : base + 256],
                    lhs1,
                    b_sb[:, 256 + 130 : 512],
                    start=True,
                    stop=True,
                )

            loop_ctx = (
                tc.For_i(0, repeats, 1) if repeats > 1 else contextlib.nullcontext()
            )
            with loop_ctx:
                for cc in range(n_images // gsz):
                    in_eng = nc.sync if (not alt_rings or cc % 2 == 0) else nc.scalar
                    out_eng = nc.scalar if (not alt_rings or cc % 2 == 0) else nc.sync
                    xt = xpool.tile([128, 512 * gsz], f16)
                    in_eng.dma_start(
                        out=xt[:].rearrange("p (c2 j w) -> p c2 j w", c2=gsz, j=2),
                        in_=x_v[cc],
                    )
                    if mode == "dmaonly":
                        out_eng.dma_start(
                            out=y_v[cc],
                            in_=xt[:].rearrange(
                                "p (c2 j w) -> p c2 j w", c2=gsz, j=2
                            ),
                        )
                        continue

                    yt = ypool.tile([128, 512 * gsz], f16)
                    if mode == "samew":
                        # perf probe: 8 matmuls/img, all with the SAME lhsT,
                        # 129-col streams. Fast (~DMA floor) iff the
                        # toolchain/HW skips reloading identical stationary
                        # weights; ~dense speed iff every matmul reloads.
                        for c2 in range(gsz):
                            xo = c2 * 512
                            xs00 = xt[:, xo : xo + 128]
                            ps1 = ps1pool.tile([128, 512], f32)
                            for ko in (0, 129, 258, 383):
                                nc.tensor.matmul(
                                    ps1[:, ko : ko + 129],
                                    xs00,
                                    bh_sb[:, 0:129],
                                    start=True,
                                    stop=True,
                                )
                            t1 = tpool.tile([128, 512], f16)
                            nc.scalar.copy(out=t1[:], in_=ps1[:])
                            ps2 = ps2pool.tile([128, 512], f32)
                            for ko in (0, 129, 258, 383):
                                nc.tensor.matmul(
                                    ps2[:, ko : ko + 129],
                                    t1[:, 0:128],
                                    bw_sb[:, 0:129],
                                    start=True,
                                    stop=True,
                                )
                            nc.vector.tensor_copy(
                                out=yt[:, c2 * 512 : (c2 + 1) * 512], in_=ps2[:]
                            )
                        out_eng.dma_start(
                            out=y_v[cc],
                            in_=yt[:].rearrange(
                                "p (c2 j w) -> p c2 j w", c2=gsz, j=2
                            ),
                        )
                        continue
                    for c2 in range(gsz):
                        xo = c2 * 512

                        # MM1: t1[w, h'] = sum_h x[h, w] * BH[h, h']
                        ps1 = ps1pool.tile([128, 512], f32)
                        for wb in range(2):
                            xs0 = xt[:, xo + wb * 128 : xo + wb * 128 + 128]
                            xs1 = xt[:, xo + 256 + wb * 128 : xo + 256 + wb * 128 + 128]
                            if mode == "band":
                                band_pass(ps1, wb * 256, xs0, xs1, bh_sb)
                            elif mode == "band3":
                                # half-band: j0 dense over all 256 cols (its
                                # band is zero beyond col 129, so this both
                                # computes cols 0..129 and zeroes the rest);
                                # j1 accumulates its nonzero range [127,256)
                                base = wb * 256
                                nc.tensor.matmul(
                                    ps1[:, base : base + 256],
                                    xs0,
                                    bh_sb[:, 0:256],
                                    start=True,
                                    stop=False,
                                    skip_group_check=True,
                                )
                                nc.tensor.matmul(
                                    ps1[:, base + 127 : base + 256],
                                    xs1,
                                    bh_sb[:, 256 + 127 : 512],
                                    start=False,
                                    stop=True,
                                    skip_group_check=True,
                                )
                            else:
                                for j, lhsT in ((0, xs0), (1, xs1)):
                                    nc.tensor.matmul(
                                        ps1[:, wb * 256 : (wb + 1) * 256],
                                        lhsT,
                                        bh_sb[:, j * 256 : (j + 1) * 256],
                                        start=(j == 0),
                                        stop=(j == 1),
                                    )

                        t1 = tpool.tile([128, 512], f16)
                        if copysplit:
                            nc.scalar.copy(out=t1[:, 0:256], in_=ps1[:, 0:256])
                            nc.vector.tensor_copy(
                                out=t1[:, 256:512], in_=ps1[:, 256:512]
                            )
                        else:
                            nc.scalar.copy(out=t1[:], in_=ps1[:])

                        # MM2: y[h', w'] = sum_w t1[w, h'] * BW[w, w']
                        # ps2[p, par*256+w'] = y[par*128+p, w'] -> v1 output
                        ps2 = ps2pool.tile([128, 512], f32)
                        for par in range(2):
                            t1s0 = t1[:, par * 128 : par * 128 + 128]
                            t1s1 = t1[:, 256 + par * 128 : 256 + par * 128 + 128]
                            if mode == "band":
                                band_pass(ps2, par * 256, t1s0, t1s1, bw_sb)
                            elif mode == "band3":
                                base = par * 256
                                nc.tensor.matmul(
                                    ps2[:, base : base + 256],
                                    t1s0,
                                    bw_sb[:, 0:256],
                                    start=True,
                                    stop=False,
                                    skip_group_check=True,
                                )
                                nc.tensor.matmul(
                                    ps2[:, base + 127 : base + 256],
                                    t1s1,
                                    bw_sb[:, 256 + 127 : 512],
                                    start=False,
                                    stop=True,
                                    skip_group_check=True,
                                )
                            else:
                                for wb, lhsT in ((0, t1s0), (1, t1s1)):
                                    nc.tensor.matmul(
                                        ps2[:, par * 256 : (par + 1) * 256],
                                        lhsT,
                                        bw_sb[:, wb * 256 : (wb + 1) * 256],
                                        start=(wb == 0),
                                        stop=(wb == 1),
                                    )

                        if copysplit:
                            nc.vector.tensor_copy(
                                out=yt[:, xo : xo + 256], in_=ps2[:, 0:256]
                            )
                            nc.scalar.copy(
                                out=yt[:, xo + 256 : xo + 512], in_=ps2[:, 256:512]
                            )
                        else:
                            nc.vector.tensor_copy(
                                out=yt[:, xo : xo + 512], in_=ps2[:]
                            )

                    out_eng.dma_start(
                        out=y_v[cc],
                        in_=yt[:].rearrange("p (c2 j w) -> p c2 j w", c2=gsz, j=2),
                    )

    nc.compile()
    return nc


def _make_bands_h16(k2):
    """Bands for the h16 pipeline, fp16.

    bh: v2 layout -- rows paired to match the row-pair input DMA layout
    (contract k = 2p+j), columns h' permuted even|odd so MM2's lhsT slice
    par picks h' = 2i+par, making ps2's partition i hold output row 2i+par
    (v2 row-pair output layout).
    bw: v1 layout, natural column order (w' ranges must be contiguous for
    the banded truncation).
    """
    u, v = _factor_kernel(k2)
    BH = _band_256(u)
    BW = _band_256(v)
    perm = np.concatenate([np.arange(0, 256, 2), np.arange(1, 256, 2)])
    bh = BH[:, perm].reshape(128, 2, 256).reshape(128, 512).astype(np.float16)
    bw = (
        BW.reshape(2, 128, 256).transpose(1, 0, 2).reshape(128, 512)
    ).astype(np.float16)
    return bh, bw


def _build_nc_h16(
    n_images,
    repeats=1,
    mode="band",
    gsz=4,
    bufs=(6, 4, 6, 3, 4),
    alt_rings=True,
    wo=260,
    lean=False,
):
    """fp16 I/O, v2 row-pair layouts on both sides (1KB DMA chunks).

    MM1 is dense (row-parity contract blocks are dense in h', no truncation
    possible), 4 matmuls x 256 cols per image: every stationary load hides
    under the previous 256-col stream.

    MM2 is banded with NO small boundary matmuls: per output-row-parity par,
    the two w-half contractions write overlapping partial sums to disjoint
    PSUM regions ([0,130) from w<128, [130,259) covering w' 127..255 from
    w>=128; both streams >=129 cols so loads stay hidden). The device
    stores 260 output columns per row; the host adds the 3-col overlap
    during reassembly (y[:, 127:130] = dev[127:130] + dev[130:133]).
    """
    import contextlib

    import concourse.bacc as bacc
    import concourse.mybir as mybir
    from concourse.tile import TileContext

    f32 = mybir.dt.float32
    f16 = mybir.dt.float16
    WO = wo  # device output row width (130 + 129 + 1 pad; 256 for lean)

    nc = bacc.Bacc("TRN2", target_bir_lowering=False)
    x = nc.dram_tensor("x", (n_images, 256, 256), f16, kind="ExternalInput")
    bh = nc.dram_tensor("bh", (128, 512), f16, kind="ExternalInput")
    bw = nc.dram_tensor("bw", (128, 512), f16, kind="ExternalInput")
    y = nc.dram_tensor("y", (n_images, 256, WO), f16, kind="ExternalOutput")

    # partition p holds row pair (2p, 2p+1): 1KB (in) / 1040B (out) chunks
    x_v = x.rearrange("(cc c2) (p j) w -> cc p c2 j w", c2=gsz, j=2)
    y_v = y.rearrange("(cc c2) (p j) w -> cc p c2 j w", c2=gsz, j=2)

    xt_b, t1_b, yt_b, ps1_b, ps2_b = bufs
    with TileContext(nc) as tc:
        with (
            tc.tile_pool(name="consts", bufs=1) as cpool,
            tc.tile_pool(name="xt", bufs=xt_b) as xpool,
            tc.tile_pool(name="t1", bufs=t1_b) as tpool,
            tc.tile_pool(name="yt", bufs=yt_b) as ypool,
            tc.tile_pool(name="ps1", bufs=ps1_b, space="PSUM") as ps1pool,
            tc.tile_pool(name="ps2", bufs=ps2_b, space="PSUM") as ps2pool,
        ):
            bh_sb = cpool.tile([128, 512], f16, tag="bh")
            bw_sb = cpool.tile([128, 512], f16, tag="bw")
            nc.sync.dma_start(out=bh_sb[:], in_=bh[:])
            nc.sync.dma_start(out=bw_sb[:], in_=bw[:])

            loop_ctx = (
                tc.For_i(0, repeats, 1) if repeats > 1 else contextlib.nullcontext()
            )
            with loop_ctx:
                for cc in range(n_images // gsz):
                    in_eng = nc.sync if (not alt_rings or cc % 2 == 0) else nc.scalar
                    out_eng = nc.scalar if (not alt_rings or cc % 2 == 0) else nc.sync
                    xt = xpool.tile([128, 512 * gsz], f16)
                    in_eng.dma_start(
                        out=xt[:].rearrange("p (c2 j w) -> p c2 j w", c2=gsz, j=2),
                        in_=x_v[cc],
                    )
                    yt = ypool.tile([128, WO * 2 * gsz], f16)
                    if mode == "dmaonly":
                        out_eng.dma_start(
                            out=y_v[cc],
                            in_=yt[:].rearrange(
                                "p (c2 j w) -> p c2 j w", c2=gsz, j=2
                            ),
                        )
                        continue

                    for c2 in range(gsz):
                        xo = c2 * 512
                        yo = c2 * (WO * 2)

                        # MM1 (dense): t1[w, h'p] = sum_h x[h, w] BH[h, h'p]
                        ps1 = ps1pool.tile([128, 512], f32)
                        for wb in range(2):
                            for j in range(2):
                                nc.tensor.matmul(
                                    ps1[:, wb * 256 : (wb + 1) * 256],
                                    xt[
                                        :,
                                        xo + j * 256 + wb * 128 : xo
                                        + j * 256
                                        + wb * 128
                                        + 128,
                                    ],
                                    bh_sb[:, j * 256 : (j + 1) * 256],
                                    start=(j == 0),
                                    stop=(j == 1),
                                )

                        t1 = tpool.tile([128, 512], f16)
                        if lean:
                            nc.scalar.copy(out=t1[:], in_=ps1[:])
                        else:
                            nc.scalar.copy(out=t1[:, 0:256], in_=ps1[:, 0:256])
                            nc.vector.tensor_copy(
                                out=t1[:, 256:512], in_=ps1[:, 256:512]
                            )

                        if lean:
                            # b16:dense structure: one ps2 tile, dense MM2,
                            # one full-width DVE evac (requires wo=256)
                            assert WO == 256 and mode == "dense"
                            ps2 = ps2pool.tile([128, 512], f32)
                            for par in range(2):
                                for wb in range(2):
                                    nc.tensor.matmul(
                                        ps2[:, par * 256 : (par + 1) * 256],
                                        t1[
                                            :,
                                            wb * 256 + par * 128 : wb * 256
                                            + par * 128
                                            + 128,
                                        ],
                                        bw_sb[:, wb * 256 : (wb + 1) * 256],
                                        start=(wb == 0),
                                        stop=(wb == 1),
                                    )
                            nc.vector.tensor_copy(
                                out=yt[:, yo : yo + 512], in_=ps2[:]
                            )
                            continue

                        # MM2 (banded, overlapping partials): for parity par,
                        # ps2[i, 0:130)   = sum_{w<128}  t1[w, h'=2i+par] BW[w, w'] (w' 0..129)
                        # ps2[i, 130:259) = sum_{w>=128} t1[w, h'=2i+par] BW[w, w'] (w' 127..255)
                        for par in range(2):
                            ps2 = ps2pool.tile([128, 512], f32)
                            if mode == "dense":
                                for wb in range(2):
                                    nc.tensor.matmul(
                                        ps2[:, 0:256],
                                        t1[
                                            :,
                                            wb * 256 + par * 128 : wb * 256
                                            + par * 128
                                            + 128,
                                        ],
                                        bw_sb[:, wb * 256 : (wb + 1) * 256],
                                        start=(wb == 0),
                                        stop=(wb == 1),
                                    )
                                yslice = yt[
                                    :, yo + par * WO : yo + par * WO + 256
                                ]
                                if par == 0:
                                    nc.scalar.copy(out=yslice, in_=ps2[:, 0:256])
                                else:
                                    nc.vector.tensor_copy(
                                        out=yslice, in_=ps2[:, 0:256]
                                    )
                                continue
                            nc.tensor.matmul(
                                ps2[:, 0:130],
                                t1[:, par * 128 : par * 128 + 128],
                                bw_sb[:, 0:130],
                                start=True,
                                stop=True,
                            )
                            nc.tensor.matmul(
                                ps2[:, 130:259],
                                t1[:, 256 + par * 128 : 256 + par * 128 + 128],
                                bw_sb[:, 256 + 127 : 512],
                                start=True,
                                stop=True,
                            )
                            yslice = yt[:, yo + par * WO : yo + par * WO + 259]
                            if par == 0:
                                nc.scalar.copy(out=yslice, in_=ps2[:, 0:259])
                            else:
                                nc.vector.tensor_copy(
                                    out=yslice, in_=ps2[:, 0:259]
                                )

                    out_eng.dma_start(
                        out=y_v[cc],
                        in_=yt[:].rearrange("p (c2 j w) -> p c2 j w", c2=gsz, j=2),
                    )

    nc.compile()
    return nc


_NC_CACHE = {}


def _build_nc(n_images, repeats=1, mode="full", layout=None, gsz=2,
              bufs=(12, 4, 8, 3, 3), alt_rings=True, swdge_in=False,
              tri=False, copysplit=False, burst=0):
    """Builds the per-core Bass module.

    gsz: images per input/output DMA (bigger transfers, fewer instructions)
    bufs: (xt, t1, yt, ps1, ps2) tile-pool buffer counts
    alt_rings: alternate in/out DMAs across both HWDGE rings (sync/scalar)
    """
    if layout is None:
        layout = LAYOUT
    import contextlib

    import concourse.bacc as bacc
    import concourse.mybir as mybir
    from concourse.tile import TileContext

    f32 = mybir.dt.float32
    f32r = mybir.dt.float32r

    nc = bacc.Bacc("TRN2", target_bir_lowering=False)
    x = nc.dram_tensor("x", (n_images, 256, 256), f32r, kind="ExternalInput")
    bh = nc.dram_tensor("bh", (128, 512), f32r, kind="ExternalInput")
    bw = nc.dram_tensor("bw", (128, 512), f32r, kind="ExternalInput")
    y = nc.dram_tensor("y", (n_images, 256, 256), f32, kind="ExternalOutput")

    if layout == "v2":
        # partition p holds rows 2p and 2p+1: 2KB-contiguous DMA lines
        x_v = x.rearrange("(cc c2) (p j) w -> cc p c2 j w", c2=gsz, j=2)
        y_v = y.rearrange("(cc c2) (p j) w -> cc p c2 j w", c2=gsz, j=2)
    else:
        # partition p holds rows p and 128+p: two 1KB chunks per image
        x_v = x.rearrange("(cc c2) (j p) w -> cc p c2 j w", c2=gsz, p=128)
        y_v = y.rearrange("(cc c2) (j p) w -> cc p c2 j w", c2=gsz, p=128)

    xt_b, t1_b, yt_b, ps1_b, ps2_b = bufs
    with TileContext(nc) as tc:
        with (
            tc.tile_pool(name="consts", bufs=1) as cpool,
            tc.tile_pool(name="xt", bufs=xt_b) as xpool,
            tc.tile_pool(name="t1", bufs=t1_b) as tpool,
            tc.tile_pool(name="yt", bufs=yt_b) as ypool,
            tc.tile_pool(name="ps1", bufs=ps1_b, space="PSUM") as ps1pool,
            tc.tile_pool(name="ps2", bufs=ps2_b, space="PSUM") as ps2pool,
        ):
            bh_sb = cpool.tile([128, 512], f32r, tag="bh")
            bw_sb = cpool.tile([128, 512], f32r, tag="bw")
            nc.sync.dma_start(out=bh_sb[:], in_=bh[:])
            nc.sync.dma_start(out=bw_sb[:], in_=bw[:])

            loop_ctx = (
                tc.For_i(0, repeats, 1) if repeats > 1 else contextlib.nullcontext()
            )
            with loop_ctx:
                pending_outs = []
                for cc in range(n_images // gsz):
                    in_eng = nc.sync if (not alt_rings or cc % 2 == 0) else nc.scalar
                    out_eng = nc.scalar if (not alt_rings or cc % 2 == 0) else nc.sync
                    if swdge_in:
                        in_eng = nc.gpsimd
                    if tri:
                        # third DGE path: SWDGE carries half the input stream
                        in_eng = nc.sync if cc % 2 == 0 else nc.gpsimd
                        out_eng = nc.scalar
                    xt = xpool.tile([128, 512 * gsz], f32r)
                    in_eng.dma_start(
                        out=xt[:].rearrange("p (c2 j w) -> p c2 j w", c2=gsz, j=2),
                        in_=x_v[cc],
                    )
                    if mode == "dmaonly":
                        out_eng.dma_start(
                            out=y_v[cc],
                            in_=xt[:]
                            .bitcast(f32)
                            .rearrange("p (c2 j w) -> p c2 j w", c2=gsz, j=2),
                        )
                        continue

                    yt = ypool.tile([128, 512 * gsz], f32)
                    for c2 in range(gsz):
                        xo = c2 * 512
                        # MM1: t1[w, h'] = sum_h x[h, w] * BH[h, h']
                        ps1 = ps1pool.tile([128, 512], f32)
                        for wb in range(2):
                            for j in range(2):
                                lhsT = xt[
                                    :,
                                    xo + j * 256 + wb * 128 : xo
                                    + j * 256
                                    + wb * 128
                                    + 128,
                                ]
                                rhs = bh_sb[:, j * 256 : (j + 1) * 256]
                                nc.tensor.matmul(
                                    ps1[:, wb * 256 : (wb + 1) * 256],
                                    lhsT,
                                    rhs,
                                    start=(j == 0),
                                    stop=(j == 1),
                                )

                        t1 = tpool.tile([128, 512], f32r)
                        if copysplit:
                            nc.scalar.copy(out=t1[:, 0:256], in_=ps1[:, 0:256])
                            nc.vector.tensor_copy(
                                out=t1[:, 256:512], in_=ps1[:, 256:512]
                            )
                        else:
                            nc.scalar.copy(out=t1[:], in_=ps1[:])

                        # MM2: y[h', w'] = sum_w t1[w, h'] * BW[w, w']
                        ps2 = ps2pool.tile([128, 512], f32)
                        for par in range(2):
                            for wb in range(2):
                                lhsT = t1[
                                    :,
                                    wb * 256 + par * 128 : wb * 256 + par * 128 + 128,
                                ]
                                rhs = bw_sb[:, wb * 256 : (wb + 1) * 256]
                                nc.tensor.matmul(
                                    ps2[:, par * 256 : (par + 1) * 256],
                                    lhsT,
                                    rhs,
                                    start=(wb == 0),
                                    stop=(wb == 1),
                                )

                        if copysplit:
                            nc.vector.tensor_copy(
                                out=yt[:, c2 * 512 : c2 * 512 + 256],
                                in_=ps2[:, 0:256],
                            )
                            nc.scalar.copy(
                                out=yt[:, c2 * 512 + 256 : (c2 + 1) * 512],
                                in_=ps2[:, 256:512],
                            )
                        else:
                            nc.vector.tensor_copy(
                                out=yt[:, c2 * 512 : (c2 + 1) * 512], in_=ps2[:]
                            )
                    if burst:
                        pending_outs.append((cc, yt))
                        if len(pending_outs) >= burst:
                            for occ, oyt in pending_outs:
                                nc.scalar.dma_start(
                                    out=y_v[occ],
                                    in_=oyt[:].rearrange(
                                        "p (c2 j w) -> p c2 j w", c2=gsz, j=2
                                    ),
                                )
                            pending_outs = []
                    else:
                        out_eng.dma_start(
                            out=y_v[cc],
                            in_=yt[:].rearrange(
                                "p (c2 j w) -> p c2 j w", c2=gsz, j=2
                            ),
                        )
                for occ, oyt in pending_outs:
                    nc.scalar.dma_start(
                        out=y_v[occ],
                        in_=oyt[:].rearrange("p (c2 j w) -> p c2 j w", c2=gsz, j=2),
                    )

    nc.compile()
    return nc


def _get_nc(n_images, repeats=1, mode="full", layout=None, **kw):
    key = (n_images, repeats, mode, layout or LAYOUT, tuple(sorted(kw.items())))
    if key not in _NC_CACHE:
        if (layout or LAYOUT) == "b16":
            _NC_CACHE[key] = _build_nc_f16(n_images, repeats, mode, **kw)
        elif (layout or LAYOUT) == "h16":
            _NC_CACHE[key] = _build_nc_h16(n_images, repeats, mode, **kw)
        else:
            _NC_CACHE[key] = _build_nc(n_images, repeats, mode, layout, **kw)
    return _NC_CACHE[key]


def _make_in_map(x_core, k2, layout=None):
    """Device input map for one core from its [C,H,W] fp32 slice."""
    if layout is None:
        layout = LAYOUT
    if layout in ("b16", "h16"):
        if layout == "b16":
            bh_sb, bw_sb = _make_bands_f16(k2)
        else:
            bh_sb, bw_sb = _make_bands_h16(k2)
        return {
            "x": np.ascontiguousarray(x_core, dtype=np.float16),
            "bh": bh_sb,
            "bw": bw_sb,
        }
    bh_sb, bw_sb = _make_bands(k2, layout)
    return {
        "x": np.ascontiguousarray(x_core, dtype=np.float32),
        "bh": bh_sb,
        "bw": bw_sb,
    }


# default device pipeline: fp16 I/O + dense 256-col matmuls (measured
# fastest: every stationary-weight load hides under a full 256-col stream;
# banded variants' short streams expose the ~256-cycle weight reloads)
MODE = os.environ.get("BLUR_MODE", "dense")


def kernel(x, kernel, _trace=False):
    from concourse import bass_utils

    x = np.asarray(x)
    k2 = np.asarray(kernel, dtype=np.float32)
    assert x.shape == (B, C, H, W), x.shape
    assert k2.shape == (KH, KW), k2.shape

    if LAYOUT in ("b16", "h16"):
        x16 = np.ascontiguousarray(x, dtype=np.float16)
        if LAYOUT == "b16":
            bh_sb, bw_sb = _make_bands_f16(k2)
        else:
            bh_sb, bw_sb = _make_bands_h16(k2)
        nc = _get_nc(C, mode=MODE, layout=LAYOUT)
        in_maps = [{"x": x16[b], "bh": bh_sb, "bw": bw_sb} for b in range(B)]
    else:
        x32 = np.ascontiguousarray(x, dtype=np.float32)
        bh_sb, bw_sb = _make_bands(k2)
        nc = _get_nc(C)
        in_maps = [{"x": x32[b], "bh": bh_sb, "bw": bw_sb} for b in range(B)]
    res = bass_utils.run_bass_kernel_spmd(
        nc, in_maps, core_ids=list(range(N_CORES)), trace=_trace
    )
    ys = [np.asarray(res.results[b]["y"], dtype=np.float32) for b in range(B)]
    if LAYOUT == "h16":
        # reassemble: device emits 260 cols/row = [130 w-lo partial | 129
        # w-hi partial | pad]; the 3-col overlap (w' 127..129) is summed here
        out = np.empty((B, C, H, W), dtype=np.float32)
        for b, yd in enumerate(ys):
            out[b, :, :, 0:127] = yd[:, :, 0:127]
            out[b, :, :, 127:130] = yd[:, :, 127:130] + yd[:, :, 130:133]
            out[b, :, :, 130:256] = yd[:, :, 133:259]
    else:
        out = np.stack(ys, axis=0)
    if _trace:
        return out, res
    return out

